# revision 1
# baseline (speedup 1.0000x reference)
"""Truncated-signature kernel (CLF_Adam_Layer) for 8x TRN2 NeuronCores.

Input  x: [8, 32, 64] fp32.  Per (batch, segment): v = -x[b, s, :],
output row = concat(v, flat(v (x) v), flat(v (x) v (x) v)) -> [8, 32, 266304].

Sharding: pure data-parallel over batch; core c computes x[c] -> [32, 266304].

Per-core dataflow (segments in pairs a=2p, b=2p+1, grouped in octets of 8):
  mmA (K=2): psA[128,64] = lhsT_pad_p.T @ vcomp_p
      rows 0:64 = v_a (x) v_a (level2 of seg a), rows 64:128 = v_b (x) v_b.
  l2 flatten: psA -> SBUF -> DRAM scratch l2dram[32, 4096] (partition-major
      DMA iteration = exactly the flattened level2), read back per octet as
      oct_t[8, 4096] (8 segments, base partition 0).
  mmB (K=8, x8 chunks): psB[128,512] = lhsT_oct[:,p*128:+128].T @ oct chunk.
      lhsT_oct is zero except rows {2(p%4), 2(p%4)+1}, so
      rows 0:64 = v_a[i] * l2flat_a[chunk c]  (level3), rows 64:128 seg b.
  PSUM->SBUF copies (DVE/ACT alternating) into outsb[128, 4096], then one
      2MB DMA per pair writes both level3 blocks (128 x 16KB contiguous).
  level2 output: one DRAM->DRAM DMA from l2dram at the end.
"""

import numpy as np

B, S, D = 8, 32, 64
PAIRS = S // 2        # 16
OCTETS = 4            # 8 segments (4 pairs) per octet
D2 = D * D            # 4096
D3 = D2 * D           # 262144
ROW = D + D2 + D3     # 266304
L2OFF = D             # 64
L3OFF = D + D2        # 4160
NCHUNK = 8            # D2 / 512 psum-bank chunks
CHUNK = D2 // NCHUNK  # 512

_compiled = None


def _build(precision="comp", stage=5, loop_n=1, big_bufs=6,
           l3_engines=("scalar", "sync"), l3_split=2, copy_eng="alt",
           rb_eng="gpsimd", use_f32r=None, l2w_eng="gpsimd", hilo_eng="gpsimd"):
    import concourse.bacc as bacc
    import concourse.mybir as mybir
    import concourse.tile as tile

    if use_f32r is not None:  # legacy flag from probes
        precision = "f32r" if use_f32r else "f32"
    f32 = mybir.dt.float32
    bf16 = mybir.dt.bfloat16
    mmdt = {"f32r": mybir.dt.float32r, "f32": mybir.dt.float32,
            "comp": bf16}[precision]

    nc = bacc.Bacc("TRN2", target_bir_lowering=False, debug=False)
    x = nc.dram_tensor("x", [S, D], f32, kind="ExternalInput").ap()
    out = nc.dram_tensor("out", [S, ROW], f32, kind="ExternalOutput").ap()

    with tile.TileContext(nc) as tc:
        with (
            tc.tile_pool(name="const", bufs=1) as cpool,
            tc.tile_pool(name="small", bufs=4) as spool,
            tc.tile_pool(name="oct", bufs=2) as opool,
            tc.tile_pool(name="big", bufs=big_bufs) as bpool,
            tc.tile_pool(name="dram", bufs=1, space="DRAM") as dpool,
            tc.tile_pool(name="psA", bufs=2, space="PSUM") as psa_pool,
            tc.tile_pool(name="psB", bufs=4, space="PSUM") as psb_pool,
        ):
            # ---- prologue: load x, negate, build packed v layouts ----
            x_s = cpool.tile([S, D], f32)
            nc.sync.dma_start(out=x_s[:], in_=x[:])
            v_s = cpool.tile([S, D], f32)
            nc.scalar.mul(v_s[:], x_s[:], -1.0)

            # level1 output: out[s, 0:64] = v_s
            nc.sync.dma_start(out=out[:, 0:D], in_=v_s[:])

            # mmA weights: lhsT_pad[0, p*128 : +64] = v_{2p}
            #              lhsT_pad[1, p*128+64 : +64] = v_{2p+1}
            lhsT_pad = cpool.tile([2, PAIRS * 128], f32)
            nc.vector.memset(lhsT_pad[:], 0.0)
            dst0 = lhsT_pad[0:1, :].rearrange("p (n c) -> p n c", c=128)[:, :, 0:D]
            nc.sync.dma_start(out=dst0, in_=v_s[0:S:2, :])
            dst1 = lhsT_pad[1:2, :].rearrange("p (n c) -> p n c", c=128)[:, :, D:128]
            nc.sync.dma_start(out=dst1, in_=v_s[1:S:2, :])

            # mmA moving: v_comp[e, p*64:(p+1)*64] = v_{2p+e}
            v_comp = cpool.tile([2, PAIRS * D], f32)
            nc.sync.dma_start(out=v_comp[0:1, :], in_=v_s[0:S:2, :])
            nc.sync.dma_start(out=v_comp[1:2, :], in_=v_s[1:S:2, :])

            # mmB weights: for pair p, rows {2(p%4), 2(p%4)+1} of
            # lhsT_oct[:, p*128:(p+1)*128] hold v_{2p} / v_{2p+1} at column
            # halves; all other rows zero.  Row r serves segments r+8t.
            lhsT_oct = cpool.tile([8, PAIRS * 128], f32)
            nc.vector.memset(lhsT_oct[:], 0.0)
            for r in range(8):
                off = (r // 2) * 128 + (r % 2) * D
                dst = lhsT_oct[r:r + 1, :].rearrange(
                    "p (t c) -> p t c", c=512)[:, :, off:off + D]
                nc.sync.dma_start(out=dst, in_=v_s[r:S:8, :])

            weights = None
            if precision == "comp":
                # hi/lo split of v in bf16: v = vh + vl + O(2^-18)
                vh_bf = cpool.tile([S, D], bf16)
                nc.vector.tensor_copy(vh_bf[:], v_s[:])
                vh_f = cpool.tile([S, D], f32)
                nc.vector.tensor_copy(vh_f[:], vh_bf[:])
                vl_f = cpool.tile([S, D], f32)
                nc.vector.tensor_sub(vl_f[:], v_s[:], vh_f[:])
                vl_bf = cpool.tile([S, D], bf16)
                nc.vector.tensor_copy(vl_bf[:], vl_f[:])
                lhsT_hi = cpool.tile([8, PAIRS * 128], bf16)
                lhsT_lo = cpool.tile([8, PAIRS * 128], bf16)
                nc.vector.memset(lhsT_hi[:], 0.0)
                nc.vector.memset(lhsT_lo[:], 0.0)
                for src_t, dst_t in ((vh_bf, lhsT_hi), (vl_bf, lhsT_lo)):
                    for r in range(8):
                        off = (r // 2) * 128 + (r % 2) * D
                        dst = dst_t[r:r + 1, :].rearrange(
                            "p (t c) -> p t c", c=512)[:, :, off:off + D]
                        nc.sync.dma_start(out=dst, in_=src_t[r:S:8, :])
                l2hi_dram = dpool.tile([S, D2], bf16)
                l2lo_dram = dpool.tile([S, D2], bf16)
                weights = (lhsT_hi, lhsT_lo, l2hi_dram, l2lo_dram)

            l2dram = dpool.tile([S, D2], f32)
            anchor = dpool.tile([64, 128], f32)

            import contextlib

            loop_cm = (
                tc.For_i(0, loop_n, 1) if loop_n > 1 else contextlib.nullcontext()
            )
            with loop_cm:
                _body(nc, tc, stage, mmdt, f32, out,
                      lhsT_pad, v_comp, lhsT_oct, l2dram, anchor,
                      spool, opool, bpool, psa_pool, psb_pool,
                      l3_engines, l3_split, copy_eng, rb_eng,
                      precision, weights, dpool, l2w_eng, hilo_eng)

    nc.compile()
    return nc


def _body(nc, tc, stage, mmdt, f32, out, lhsT_pad, v_comp, lhsT_oct,
          l2dram, anchor, spool, opool, bpool, psa_pool, psb_pool,
          l3_engines=("sync", "scalar"), l3_split=1, copy_eng="alt",
          rb_eng="sync", precision="f32r", weights=None, dpool=None,
          l2w_eng="gpsimd", hilo_eng="gpsimd"):
    import concourse.mybir as mybir
    bf16 = mybir.dt.bfloat16
    dma_i = [0]

    def next_eng():
        e = getattr(nc, l3_engines[dma_i[0] % len(l3_engines)])
        dma_i[0] += 1
        return e

    if True:
        if True:
            for t in range(OCTETS):
                if stage < 1:
                    break
                # ---- level2 for this octet's 4 pairs ----
                for pp in range(4):
                    p = 4 * t + pp
                    a = 2 * p
                    psA = psa_pool.tile([128, D], f32)
                    nc.tensor.matmul(
                        psA[:],
                        lhsT_pad[:, p * 128:(p + 1) * 128],
                        v_comp[:, p * D:(p + 1) * D],
                        start=True, stop=True,
                    )
                    l2sb = spool.tile([128, D], f32)
                    nc.scalar.copy(l2sb[:], psA[:])
                    # partition-major iteration == flattened level2
                    getattr(nc, l2w_eng).dma_start(
                        out=l2dram[a:a + 2, :], in_=l2sb[:])
                    if precision == "comp":
                        lhsT_hi, lhsT_lo, l2hi_dram, l2lo_dram = weights
                        gh_bf = spool.tile([128, D], bf16, tag="gh_bf")
                        nc.vector.tensor_copy(gh_bf[:], l2sb[:])
                        gh_f = spool.tile([128, D], f32, tag="gh_f")
                        nc.scalar.copy(gh_f[:], gh_bf[:])
                        gl_bf = spool.tile([128, D], bf16, tag="gl_bf")
                        nc.vector.tensor_sub(gl_bf[:], l2sb[:], gh_f[:])
                        getattr(nc, hilo_eng).dma_start(
                            out=l2hi_dram[a:a + 2, :], in_=gh_bf[:])
                        getattr(nc, hilo_eng).dma_start(
                            out=l2lo_dram[a:a + 2, :], in_=gl_bf[:])

                if stage < 2:
                    continue
                # read back as [8 segments, 4096] at base partition 0
                if precision == "comp":
                    lhsT_hi, lhsT_lo, l2hi_dram, l2lo_dram = weights
                    oct_hi = opool.tile([8, D2], bf16, tag="oct_hi")
                    getattr(nc, rb_eng).dma_start(
                        out=oct_hi[:], in_=l2hi_dram[8 * t:8 * t + 8, :])
                    oct_lo = opool.tile([8, D2], bf16, tag="oct_lo")
                    getattr(nc, rb_eng).dma_start(
                        out=oct_lo[:], in_=l2lo_dram[8 * t:8 * t + 8, :])
                    oct_t = None
                else:
                    oct_t = opool.tile([8, D2], f32)
                    getattr(nc, rb_eng).dma_start(
                        out=oct_t[:], in_=l2dram[8 * t:8 * t + 8, :])

                # ---- level3 for this octet ----
                if stage < 3:
                    continue
                for pp in range(4):
                    p = 4 * t + pp
                    a, b = 2 * p, 2 * p + 1
                    lhsT_p = lhsT_oct[:, p * 128:(p + 1) * 128]
                    outsb = bpool.tile([128, D2], f32)
                    for c in range(NCHUNK):
                        psB = psb_pool.tile([128, CHUNK], f32)
                        if precision == "comp":
                            lhsT_hi, lhsT_lo = weights[0], weights[1]
                            wh = lhsT_hi[:, p * 128:(p + 1) * 128]
                            wl = lhsT_lo[:, p * 128:(p + 1) * 128]
                            ch = oct_hi[:, c * CHUNK:(c + 1) * CHUNK]
                            cl = oct_lo[:, c * CHUNK:(c + 1) * CHUNK]
                            nc.tensor.matmul(psB[:], wh, ch,
                                             start=True, stop=False)
                            nc.tensor.matmul(psB[:], wh, cl,
                                             start=False, stop=False)
                            nc.tensor.matmul(psB[:], wl, ch,
                                             start=False, stop=True)
                        else:
                            nc.tensor.matmul(
                                psB[:],
                                lhsT_p.bitcast(mmdt),
                                oct_t[:, c * CHUNK:(c + 1) * CHUNK].bitcast(
                                    mmdt),
                                start=True, stop=True,
                            )
                        dst = outsb[:, c * CHUNK:(c + 1) * CHUNK]
                        use_dve = (c % 2 == 0) if copy_eng == "alt" else (
                            copy_eng == "dve")
                        if use_dve:
                            nc.vector.tensor_copy(dst, psB[:])
                        else:
                            nc.scalar.copy(dst, psB[:])

                    # both segments' level3: 128 partitions x 16KB contiguous
                    if stage >= 5:
                        if l3_split == 1:
                            next_eng().dma_start(
                                out=out[a:b + 1, L3OFF:ROW], in_=outsb[:])
                        elif l3_split == 2:
                            next_eng().dma_start(
                                out=out[a, L3OFF:ROW], in_=outsb[0:64, :])
                            next_eng().dma_start(
                                out=out[b, L3OFF:ROW], in_=outsb[64:128, :])
                        elif l3_split == -2:
                            # per-segment halves, contiguous dests (4 DMAs)
                            H = 32 * D2  # 32 i-rows worth of floats
                            for row, base in ((a, 0), (b, 64)):
                                next_eng().dma_start(
                                    out=out[row, L3OFF:L3OFF + H],
                                    in_=outsb[base:base + 32, :])
                                next_eng().dma_start(
                                    out=out[row, L3OFF + H:ROW],
                                    in_=outsb[base + 32:base + 64, :])
                        else:
                            # split the free dim: full 128 partitions per DMA
                            n = l3_split
                            w = D2 // n
                            l3v = out[a:b + 1, L3OFF:ROW].rearrange(
                                "r (i n) -> r i n", n=D2)
                            for k in range(n):
                                next_eng().dma_start(
                                    out=l3v[:, :, k * w:(k + 1) * w],
                                    in_=outsb[:, k * w:(k + 1) * w])
                    else:
                        # keep copies live without the big write
                        nc.sync.dma_start(
                            out=anchor[4 * (p % 4):4 * (p % 4) + 4, :],
                            in_=outsb[0:4, 0:128])

            # level2 output, all segments at once (DRAM->DRAM)
            if stage >= 1:
                nc.gpsimd.dma_start(out=out[:, L2OFF:L3OFF], in_=l2dram[:])


def _get_compiled():
    global _compiled
    if _compiled is None:
        _compiled = _build()
    return _compiled


def kernel(x: np.ndarray) -> np.ndarray:
    from concourse.bass_utils import run_bass_kernel_spmd

    assert x.shape == (B, S, D), x.shape
    nc = _get_compiled()
    x = np.ascontiguousarray(x, dtype=np.float32)
    in_maps = [{"x": x[c]} for c in range(B)]
    res = run_bass_kernel_spmd(nc, in_maps, list(range(B)))
    return np.stack([res.results[c]["out"] for c in range(B)], axis=0)



# revision 13
# speedup vs baseline: 40.9534x; 40.9534x over previous
"""Truncated-signature kernel (CLF_Adam_Layer) for 8x TRN2 NeuronCores.

Input  x: [8, 32, 64] fp32.  Per (batch, segment): v = -x[b, s, :],
output row = concat(v, flat(v (x) v), flat(v (x) v (x) v)) -> [8, 32, 266304].

Sharding: pure data-parallel over batch; core c computes x[c] -> [32, 266304].

Per-core dataflow (segments in pairs a=2p, b=2p+1, fully per-pair pipelined):
  mmA (K=2, f32): psA[128,64] = lhsT_pad_p.T @ vcomp_p
      rows 0:64 = v_a (x) v_a (level2 of seg a), rows 64:128 = v_b (x) v_b.
  reshape: psA -> SBUF -> bf16 cast -> DRAM scratch l2bf_dram[a:a+2, :]
      (partition-major iteration == the flattened level2), read straight
      back as pair_bf[2, 4096] (seg per partition, 8KB contiguous rows).
  mmB (K=2, single bf16 matmul, x8 chunks): psB[128,512] =
      lhsT_bf[:,p*128:+128].T @ pair_bf chunk, so rows 0:64 =
      v_a[i] * l2flat_a[chunk c] (level3), rows 64:128 seg b.
      Error ~2^-8 (bf16 rounding of both factors; elements are pure products
      so no cancellation) vs 2e-2 tolerance.
  PSUM->SBUF copies (DVE/ACT alternating) into outsb[128, 4096], then two
      1MB flat-dest DMAs per pair write the level3 blocks (64 x 16KB each;
      flat dests spray across all 16 SDMA engines, 2-row dests do not).
  level2 output: one batched DRAM->DRAM cast DMA (bf16 -> f32) at the end
      of issue order; executes while mmB stream drains.

Measured (NTFF profile, core 0): ~137us vs ~212us for the previous
hi/lo-compensated octet kernel; HBM write wall is ~319 GB/s/NC so the
33.5MB/core output stream floors at ~105us + ~15us ramp/drain.
"""

import numpy as np

B, S, D = 8, 32, 64
PAIRS = S // 2        # 16
D2 = D * D            # 4096
D3 = D2 * D           # 262144
ROW = D + D2 + D3     # 266304
L2OFF = D             # 64
L3OFF = D + D2        # 4160
NCHUNK = 8            # D2 / 512 psum-bank chunks
CHUNK = D2 // NCHUNK  # 512

_compiled = None


def _build(big_bufs=7, l3_engines=("scalar", "sync"), l3_split=2,
           copy_eng="alt", scratch_eng="gpsimd", l2cast_eng="gpsimd",
           psb_bufs=4, pair_bufs=8, reshape="dram", resh_eng="sync",
           spool_bufs=8, psa_bufs=3, early_pairs=1, late_pairs=0,
           direct_cast=False, pair0_sb2sb=False):
    import concourse.bacc as bacc
    import concourse.mybir as mybir
    import concourse.tile as tile

    f32 = mybir.dt.float32
    bf16 = mybir.dt.bfloat16

    nc = bacc.Bacc("TRN2", target_bir_lowering=False, debug=False)
    x = nc.dram_tensor("x", [S, D], f32, kind="ExternalInput").ap()
    out = nc.dram_tensor("out", [S, ROW], f32, kind="ExternalOutput").ap()

    with tile.TileContext(nc) as tc:
        with (
            tc.tile_pool(name="const", bufs=1) as cpool,
            tc.tile_pool(name="small", bufs=spool_bufs) as spool,
            tc.tile_pool(name="pair", bufs=pair_bufs) as ppool,
            tc.tile_pool(name="big", bufs=big_bufs) as bpool,
            tc.tile_pool(name="dram", bufs=1, space="DRAM") as dpool,
            tc.tile_pool(name="psA", bufs=psa_bufs, space="PSUM") as psa_pool,
            tc.tile_pool(name="psB", bufs=psb_bufs, space="PSUM") as psb_pool,
        ):
            # ---- prologue: load x, negate, build packed v layouts ----
            x_s = cpool.tile([S, D], f32)
            nc.sync.dma_start(out=x_s[:], in_=x[:])
            v_s = cpool.tile([S, D], f32)
            nc.scalar.mul(v_s[:], x_s[:], -1.0)

            # level1 output: out[s, 0:64] = v_s
            nc.sync.dma_start(out=out[:, 0:D], in_=v_s[:])

            # mmA/mmB weights: lhsT_pad[0, p*128 : +64] = v_{2p}
            #                  lhsT_pad[1, p*128+64 : +64] = v_{2p+1}
            lhsT_pad = cpool.tile([2, PAIRS * 128], f32)
            nc.vector.memset(lhsT_pad[:], 0.0)
            dst0 = lhsT_pad[0:1, :].rearrange("p (n c) -> p n c", c=128)[:, :, 0:D]
            nc.sync.dma_start(out=dst0, in_=v_s[0:S:2, :])
            dst1 = lhsT_pad[1:2, :].rearrange("p (n c) -> p n c", c=128)[:, :, D:128]
            nc.sync.dma_start(out=dst1, in_=v_s[1:S:2, :])
            # bf16 twin for mmB (zeros cast to zeros)
            lhsT_bf = cpool.tile([2, PAIRS * 128], bf16)
            nc.vector.tensor_copy(lhsT_bf[:], lhsT_pad[:])

            # mmA moving: v_comp[e, p*64:(p+1)*64] = v_{2p+e}
            v_comp = cpool.tile([2, PAIRS * D], f32)
            nc.sync.dma_start(out=v_comp[0:1, :], in_=v_s[0:S:2, :])
            nc.sync.dma_start(out=v_comp[1:2, :], in_=v_s[1:S:2, :])

            l2bf_dram = (dpool.tile([S, D2], bf16, name="l2bf_dram")
                         if reshape == "dram" else None)
            pair0_bf = (cpool.tile([2, D2], bf16, name="pair0_bf")
                        if pair0_sb2sb else None)
            # rotating 16-partition window: pair p uses rows 2*(p%8)..+2.
            # Dest partitions 0-15 spread the SBUF->SBUF reshape DMAs over
            # SDMA engines 0/2/4/6; 8 windows of slack before WAR reuse.
            win_bf = (cpool.tile([16, D2], bf16, name="win_bf")
                      if reshape == "sb2sb" else None)

            dma_i = [0]

            def next_eng():
                e = getattr(nc, l3_engines[dma_i[0] % len(l3_engines)])
                dma_i[0] += 1
                return e

            for p in range(PAIRS):
                a, b = 2 * p, 2 * p + 1
                # ---- level2 for this pair ----
                psA = psa_pool.tile([128, D], f32)
                nc.tensor.matmul(
                    psA[:],
                    lhsT_pad[:, p * 128:(p + 1) * 128],
                    v_comp[:, p * D:(p + 1) * D],
                    start=True, stop=True,
                )
                l2bf = spool.tile([128, D], bf16, tag="l2bf")
                if direct_cast:
                    # PSUM -> SBUF with bf16 cast in one DVE op
                    nc.vector.tensor_copy(l2bf[:], psA[:])
                else:
                    l2sb = spool.tile([128, D], f32)
                    nc.scalar.copy(l2sb[:], psA[:])
                    nc.vector.tensor_copy(l2bf[:], l2sb[:])
                if pair0_sb2sb and p == 0:
                    # single-hop reshape for the ramp-critical first pair
                    pair_bf = pair0_bf[:]
                    nc.sync.dma_start(out=pair_bf, in_=l2bf[:])
                    # still write scratch so the batched level2 cast works
                    getattr(nc, scratch_eng).dma_start(
                        out=l2bf_dram[a:a + 2, :], in_=l2bf[:])
                elif reshape == "dram":
                    getattr(nc, scratch_eng).dma_start(
                        out=l2bf_dram[a:a + 2, :], in_=l2bf[:])
                    # read straight back: seg-per-partition rows, 8KB runs
                    pair_bf = ppool.tile([2, D2], bf16)
                    getattr(nc, scratch_eng).dma_start(
                        out=pair_bf[:], in_=l2bf_dram[a:a + 2, :])
                else:
                    w = 2 * (p % 8)
                    pair_bf = win_bf[w:w + 2, :]
                    getattr(nc, resh_eng).dma_start(out=pair_bf, in_=l2bf[:])
                    # level2 output rows, cast bf16 -> f32 during DMA
                    getattr(nc, l2cast_eng).dma_start(
                        out=out[a:a + 2, L2OFF:L3OFF], in_=pair_bf)

                # ---- level3 for this pair (K=2 bf16 matmul) ----
                wp = lhsT_bf[:, p * 128:(p + 1) * 128]
                outsb = bpool.tile([128, D2], f32)
                for c in range(NCHUNK):
                    psB = psb_pool.tile([128, CHUNK], f32)
                    nc.tensor.matmul(
                        psB[:], wp,
                        pair_bf[:, c * CHUNK:(c + 1) * CHUNK],
                        start=True, stop=True,
                    )
                    dst = outsb[:, c * CHUNK:(c + 1) * CHUNK]
                    use_dve = (c % 2 == 0) if copy_eng == "alt" else (
                        copy_eng == "dve")
                    if use_dve:
                        nc.vector.tensor_copy(dst, psB[:])
                    else:
                        nc.scalar.copy(dst, psB[:])

                # both segments' level3: flat per-row dests, 64 x 16KB each
                if p < early_pairs or p >= PAIRS - late_pairs:
                    # ramp trim: stream this pair's output per chunk so the
                    # write queues start draining ~5us earlier
                    ov = out[a, L3OFF:ROW].rearrange("(i m) -> i m", m=D2)
                    ovb = out[b, L3OFF:ROW].rearrange("(i m) -> i m", m=D2)
                    for c in range(NCHUNK):
                        cs = slice(c * CHUNK, (c + 1) * CHUNK)
                        next_eng().dma_start(
                            out=ov[:, cs], in_=outsb[0:64, cs])
                        next_eng().dma_start(
                            out=ovb[:, cs], in_=outsb[64:128, cs])
                elif l3_split == 2:
                    next_eng().dma_start(
                        out=out[a, L3OFF:ROW], in_=outsb[0:64, :])
                    next_eng().dma_start(
                        out=out[b, L3OFF:ROW], in_=outsb[64:128, :])
                else:
                    # per-segment halves, contiguous dests (4 DMAs)
                    H = 32 * D2
                    for row, base in ((a, 0), (b, 64)):
                        next_eng().dma_start(
                            out=out[row, L3OFF:L3OFF + H],
                            in_=outsb[base:base + 32, :])
                        next_eng().dma_start(
                            out=out[row, L3OFF + H:ROW],
                            in_=outsb[base + 32:base + 64, :])

            if reshape == "dram":
                # level2 output, all segments: DRAM->DRAM cast bf16 -> f32,
                # overlaps the remaining mmB/DMA stream
                getattr(nc, l2cast_eng).dma_start(
                    out=out[:, L2OFF:L3OFF], in_=l2bf_dram[:])

    nc.compile()
    return nc


def _get_compiled():
    global _compiled
    if _compiled is None:
        _compiled = _build()
    return _compiled


def kernel(x: np.ndarray) -> np.ndarray:
    from concourse.bass_utils import run_bass_kernel_spmd

    assert x.shape == (B, S, D), x.shape
    nc = _get_compiled()
    x = np.ascontiguousarray(x, dtype=np.float32)
    in_maps = [{"x": x[c]} for c in range(B)]
    res = run_bass_kernel_spmd(nc, in_maps, list(range(B)))
    return np.stack([res.results[c]["out"] for c in range(B)], axis=0)
